# revision 15
# baseline (speedup 1.0000x reference)
"""Trainium2 Bass kernel for nn_KnowledgeCircuit (moe_routing).

  h   = einsum('bsd,ndr,bsn->bsr', x, feature_know, feature_know_w)
  out = einsum('bsr,bsn,nrd->bsd', h, restore_know_w, restore_know)

Shapes: B=4, S=2048, D=1024, N=64, R=128.

Sharding: data-parallel over the B*S = 8192 tokens -> 1024 tokens per
NeuronCore across 8 cores; the neuron pools (fk, rk) are replicated.
No collectives.

v2 design notes (vs v1 baseline at 636us sim):
  - All matmul operands in bf16 (host-side cast; measured end-to-end
    rel err 3.9e-3 vs the 2e-2 gate). PE rate is identical to f32r
    (1 cycle/row) but DMA bytes halve and DVE ops on bf16 are 2x.
  - Host-side layout prep: x pre-transposed to [128, DK, T] (kills 64
    PE transposes + copies), w2 pre-transposed to [N, T] for the
    partition-broadcast source, fk packed per-quad, rk packed per
    4-pool group. Output is written in device layout [th, dk, 128d,
    512t] and transposed back on the host.
  - Few, large DMAs (~82 total vs ~730): the shared HWDGE device costs
    ~625ns per DMA and was the v1 bottleneck (SP track 96.5% busy).
  - rk is fully resident in SBUF (128KB/partition budget holds), so
    stage 2 reads it once; bc/g tiles stream per 4-pool group.

Per-core program:
  phase 0: one DMA each for xT halves, w1; memset h.
  phase 1: per quad of 4 pools: one fkq DMA; 2 token-groups x
           (8 dk x 4 tt) matmuls accumulate psum[t128, 4x128]; DVE
           scalar_tensor_tensor applies w1 and accumulates h[t, r] in
           f32. rk group DMAs are interleaved here to prefetch.
  phase 1.5: PE-transpose h -> hT [r, t] (bf16).
  phase 2: for each token half th and 4-pool group: one bc DMA
           (partition-broadcast of 4 w2T rows), g = hT * bc (DVE,
           bf16), 8 matmuls per pool accumulate psum[d128, t512] over
           all 64 pools; drain via scalar-engine copy + one DMA per
           (th, dk) in d-major layout.
"""

from contextlib import ExitStack

import numpy as np
import ml_dtypes

import concourse.mybir as mybir
import concourse.tile as tile
from concourse import bacc
from concourse.bass_utils import run_bass_kernel_spmd
from concourse.masks import make_identity

F32 = mybir.dt.float32
BF16 = mybir.dt.bfloat16
MULT = mybir.AluOpType.mult
ADD = mybir.AluOpType.add
COPY = mybir.ActivationFunctionType.Copy

B, S, D, N, R = 4, 2048, 1024, 64, 128
N_CORES = 8
T = B * S // N_CORES  # tokens per core

NP_BF16 = ml_dtypes.bfloat16


def build_kernel(T=1024, D=1024, N=64, R=128, debug=False):
    """Build the per-core Bass program. T tokens per core."""
    assert T == 1024 and D == 1024 and N == 64 and R == 128
    TT = T // 128          # token tiles (8)
    DK = D // 128          # d tiles (8)
    NQ = N // 4            # stage-1 quads (16)
    NG = N // 4            # stage-2 4-pool groups (16)

    nc = bacc.Bacc(None, target_bir_lowering=False, debug=debug)

    # host-prepped layouts (see _shard_inputs)
    xT_d = nc.dram_tensor("xT", [128, DK, T], BF16, kind="ExternalInput")
    w1_d = nc.dram_tensor("w1", [128, TT, N], F32, kind="ExternalInput")
    w2T_d = nc.dram_tensor("w2T", [N, T], BF16, kind="ExternalInput")
    fk_d = nc.dram_tensor("fkp", [128, NQ, DK, 4, R], BF16, kind="ExternalInput")
    rk_d = nc.dram_tensor("rkp", [NG, 128, 4, D], BF16, kind="ExternalInput")
    out_d = nc.dram_tensor("out", [2, DK, 128, T // 2], BF16, kind="ExternalOutput")

    with tile.TileContext(nc) as tc, ExitStack() as ctx:
        sb_const = ctx.enter_context(tc.tile_pool(name="const", bufs=1))
        sb_xT = ctx.enter_context(tc.tile_pool(name="xT", bufs=1))
        sb_w1 = ctx.enter_context(tc.tile_pool(name="w1p", bufs=1))
        sb_h = ctx.enter_context(tc.tile_pool(name="h", bufs=TT))
        sb_hT = ctx.enter_context(tc.tile_pool(name="hT", bufs=1))
        sb_fk = ctx.enter_context(tc.tile_pool(name="fkp", bufs=2))
        sb_rk = ctx.enter_context(tc.tile_pool(name="rkp", bufs=NG))
        sb_g = ctx.enter_context(tc.tile_pool(name="gp", bufs=3))
        sb_bc = ctx.enter_context(tc.tile_pool(name="bcp", bufs=2))
        sb_ot = ctx.enter_context(tc.tile_pool(name="otp", bufs=8))
        psum = ctx.enter_context(tc.tile_pool(name="ps", bufs=8, space="PSUM"))

        ident = sb_const.tile([128, 128], F32, tag="ident")
        make_identity(nc, ident[:])

        # ---- phase 0: bulk loads ----
        # Issue order matters: the first quad's matmuls need xT tokens
        # 0-511 and fkq[0]; everything else can trail behind them on
        # the (serialized) DMA engines.
        # interleave the first xT/fkq0 chunks so PE can start at ~3.5us
        # and never stalls waiting for the next token tile's columns
        xT = sb_xT.tile([128, DK, T], BF16, tag="xT")
        fkq0 = sb_fk.tile([128, DK, 4, R], BF16, tag="fk")
        w1s = sb_w1.tile([128, TT, N], F32, tag="w1")
        nc.sync.dma_start(xT[:, :, 0:128], xT_d[:, :, 0:128])
        nc.sync.dma_start(fkq0[:, 0:4], fk_d[:, 0, 0:4])
        nc.sync.dma_start(xT[:, :, 128:256], xT_d[:, :, 128:256])
        nc.sync.dma_start(fkq0[:, 4:DK], fk_d[:, 0, 4:DK])
        nc.sync.dma_start(xT[:, :, 256 : T // 2], xT_d[:, :, 256 : T // 2])
        nc.sync.dma_start(w1s[:], w1_d[:])
        nc.sync.dma_start(xT[:, :, T // 2 : T], xT_d[:, :, T // 2 : T])

        h = [sb_h.tile([128, R], F32, tag="h", name=f"h{i}") for i in range(TT)]
        for tt in range(TT):
            nc.gpsimd.memset(h[tt][:], 0.0)

        # rk resident tiles; DMAs interleaved with fkq loads below
        rk4 = [
            sb_rk.tile([128, 4, D], BF16, tag="rk", name=f"rk{g}") for g in range(NG)
        ]

        # ---- phase 1: h[t, r] accumulation over all pools ----
        for q in range(NQ):
            if q == 0:
                fkq = fkq0
            else:
                fkq = sb_fk.tile([128, DK, 4, R], BF16, tag="fk")
                nc.sync.dma_start(fkq[:], fk_d[:, q])
            nc.sync.dma_start(rk4[q][:], rk_d[q])
            # quad 0 runs token tiles singly so the first matmuls only
            # gate on the small xT[0:128]/fkq0 startup chunks; the last
            # quad runs pairs so its final stt drain (which gates the
            # h->hT transposes) is short
            if q == 0:
                tgroups = [[tt] for tt in range(TT)]
            elif q == NQ - 1:
                tgroups = [[2 * tg, 2 * tg + 1] for tg in range(TT // 2)]
            else:
                tgroups = [list(range(tg * 4, tg * 4 + 4)) for tg in range(TT // 4)]
            for tts in tgroups:
                hps = {
                    tt: psum.tile([128, 4, R], F32, tag="ps", name=f"hps{tt}")
                    for tt in tts
                }
                for dk in range(DK):
                    for tt in tts:
                        nc.tensor.matmul(
                            hps[tt][:],
                            xT[:, dk, tt * 128 : (tt + 1) * 128],
                            fkq[:, dk],
                            start=(dk == 0),
                            stop=(dk == DK - 1),
                        )
                for tt in tts:
                    # note: stt reads PSUM, so only DVE can run it
                    # (GPSIMD cannot access PSUM on real HW)
                    for i in range(4):
                        n = q * 4 + i
                        nc.vector.scalar_tensor_tensor(
                            h[tt][:],
                            hps[tt][:, i],
                            w1s[:, tt, n : n + 1],
                            h[tt][:],
                            MULT,
                            ADD,
                        )

        # ---- phase 1.5: hT (bf16) ----
        hT = sb_hT.tile([128, T], BF16, tag="hT")
        for tt in range(TT):
            tp = psum.tile([128, 128], F32, tag="ps")
            nc.tensor.transpose(tp[:], h[tt][:], ident[:])
            nc.scalar.activation(hT[:, tt * 128 : (tt + 1) * 128], tp[:], COPY)

        # ---- phase 2: out accumulation over all pools, token halves ----
        t5 = T // 2  # 512
        for th in range(2):
            ops = [
                psum.tile([128, t5], F32, tag="ps", name=f"ops{th}_{i}")
                for i in range(DK)
            ]
            for g4 in range(NG):
                bc = sb_bc.tile([128, 4, t5], BF16, tag="bc")
                nc.sync.dma_start(
                    bc[:],
                    w2T_d[
                        g4 * 4 : (g4 + 1) * 4, th * t5 : (th + 1) * t5
                    ].partition_broadcast(128),
                )
                for j in range(4):
                    n = g4 * 4 + j
                    g = sb_g.tile([128, t5], BF16, tag="g")
                    nc.vector.tensor_mul(
                        g[:], hT[:, th * t5 : (th + 1) * t5], bc[:, j]
                    )
                    for dk in range(DK):
                        nc.tensor.matmul(
                            ops[dk][:],
                            rk4[g4][:, j, dk * 128 : (dk + 1) * 128],
                            g[:],
                            start=(n == 0),
                            stop=(n == N - 1),
                        )
            for dk in range(DK):
                ot = sb_ot.tile([128, t5], BF16, tag="ot")
                # alternate engines so the final drain isn't serialized
                if dk % 2 == 0:
                    nc.scalar.activation(ot[:], ops[dk][:], COPY)
                else:
                    nc.vector.tensor_copy(ot[:], ops[dk][:])
                nc.sync.dma_start(out_d[th, dk], ot[:])

    nc.compile()
    return nc


_NC_CACHE = {}


def _get_nc():
    if "nc" not in _NC_CACHE:
        _NC_CACHE["nc"] = build_kernel(T=T, D=D, N=N, R=R, debug=False)
    return _NC_CACHE["nc"]


def _shard_inputs(x, feature_know_w, restore_know_w, feature_know, restore_know):
    TT, DK, NQ, NG = T // 128, D // 128, N // 4, N // 4
    xf = np.asarray(x, dtype=np.float32).reshape(B * S, D)
    w1f = np.asarray(feature_know_w, dtype=np.float32).reshape(B * S, N)
    w2f = np.asarray(restore_know_w, dtype=np.float32).reshape(B * S, N)
    fk = np.asarray(feature_know, dtype=np.float32)
    rk = np.asarray(restore_know, dtype=np.float32)

    # fk packed: fkp[p, q, dk, i, r] = fk[4q+i, 128dk+p, r], bf16
    fkp = np.ascontiguousarray(
        fk.astype(NP_BF16)
        .reshape(NQ, 4, DK, 128, R)
        .transpose(3, 0, 2, 1, 4)
    )
    # rk packed: rkp[g, p, j, d] = rk[4g+j, p, d], bf16
    rkp = np.ascontiguousarray(
        rk.astype(NP_BF16).reshape(NG, 4, R, D).transpose(0, 2, 1, 3)
    )

    in_maps = []
    for c in range(N_CORES):
        sl = slice(c * T, (c + 1) * T)
        # xT[p, dk, t] = x[t, 128dk+p] (bf16)
        xTc = np.ascontiguousarray(
            xf[sl].astype(NP_BF16).T.reshape(DK, 128, T).transpose(1, 0, 2)
        )
        # w1[p, tt, n] = w1[128tt+p, n] (f32)
        w1c = np.ascontiguousarray(
            w1f[sl].reshape(TT, 128, N).transpose(1, 0, 2)
        )
        # w2T[n, t] = w2[t, n] (bf16)
        w2Tc = np.ascontiguousarray(w2f[sl].astype(NP_BF16).T)
        in_maps.append(
            {"xT": xTc, "w1": w1c, "w2T": w2Tc, "fkp": fkp, "rkp": rkp}
        )
    return in_maps


def run(in_maps, **kwargs):
    nc = _get_nc()
    return run_bass_kernel_spmd(nc, in_maps, core_ids=list(range(N_CORES)), **kwargs)


def _unshard_out(core_out):
    # core_out: [2, DK, 128, 512] bf16 device layout -> [T, D] f32 token-major
    return core_out.astype(np.float32).transpose(0, 3, 1, 2).reshape(T, D)


def kernel(x, feature_know_w, restore_know_w, feature_know, restore_know, **_):
    in_maps = _shard_inputs(
        x, feature_know_w, restore_know_w, feature_know, restore_know
    )
    res = run(in_maps)
    out = np.concatenate([_unshard_out(r["out"]) for r in res.results], axis=0)
    return out.reshape(B, S, D)
